# revision 5
# baseline (speedup 1.0000x reference)
"""AdaptiveGraphConv Trainium2 kernel, data-parallel over batch on 8 NeuronCores.

Reference computation (per full input):
  sim  = relu(E @ E^T)                               [N, N]
  d[n] = 1 + softmax(sim, axis=1)[n, n]              (diag gate)
  Ew   = einsum('nd,dcf->ncf', E, W)                 per-node weights
  eb   = E @ bias                                    per-node bias [N, F]
  y[b,t,n,f] = (d[n] * x[b,t,n,:]) @ Ew[n] + eb[n]

Device strategy per core (2 of 16 batches, R = 2*288 = 576 rows):
  - x/y stream through the chip in fp16 (tolerance is 2e-2; fp16 error is
    ~1e-3) — halves HBM traffic, which is the roofline for this kernel.
  - host supplies x as [128, PAIRS*R] fp16: partition q = (parity, c) of a
    node pair, column p*R + r. Any run of pairs is per-partition contiguous,
    so one 8-pair DMA moves 9216B/partition in single descriptors (small
    2304B packets measurably cap SDMA engines at ~19B/ns; big ones ~27).
  - on-chip prep: d = 1 + softmax diag (diag(E E^T) = rowsum(E^2), no mask
    needed), E' = diag(d) @ E, then block-diagonal per-pair stationary
    weights Ew in fp16 and pair-stacked bias ebT2.
  - main loop over 13 tiles of nb=8 pairs: per pair 2 fp16 matmuls
    [128x128]@[128x288] into PSUM, then PSUM->SBUF bias-add written in
    place over the x tile, rotated across Vector/GpSimd/Scalar so no one
    engine bottlenecks. Loads+stores ride ONE HWDGE ring (sync) so HBM
    traffic stays unidirectional per burst; const/weight loads ride the
    scalar HWDGE ring concurrently (read/read mixing is fine).
  - host un-permutes y^T shards back to [B, T, N, F] and casts to fp32.
"""

import sys

sys.path.insert(0, "/opt/trn_rl_repo")

from contextlib import ExitStack

import numpy as np

N_CORES = 8
NODE = 207
NODE_P = 208  # padded to even node count
PAIRS = NODE_P // 2  # 104
EMB = 128
C = 64
F = 64
B = 16
T = 288
B_SH = B // N_CORES  # 2
R = B_SH * T  # 576 rows per core
RH = R // 2  # 288, matmul free-dim chunk
NB = 8  # pairs per DMA / compute tile
NT = PAIRS // NB  # 13 tiles
PRE = 5  # load prefetch depth (tiles)

_CACHE = {}


def _build():
    import concourse.tile as tile
    from concourse import bacc, mybir

    f32 = mybir.dt.float32
    f16 = mybir.dt.float16
    AF = mybir.ActivationFunctionType
    ALU = mybir.AluOpType
    AX = mybir.AxisListType

    nc = bacc.Bacc("TRN2", target_bir_lowering=False, debug=False, num_devices=N_CORES)
    xt = nc.dram_tensor("xt", [128, PAIRS * R], f16, kind="ExternalInput").ap()
    emb = nc.dram_tensor("emb", [NODE_P, EMB], f32, kind="ExternalInput").ap()
    w = nc.dram_tensor("w", [EMB, F * C], f16, kind="ExternalInput").ap()
    bias_d = nc.dram_tensor("bias", [EMB, F], f32, kind="ExternalInput").ap()
    ident = nc.dram_tensor("ident", [128, 128], f32, kind="ExternalInput").ap()
    yt = nc.dram_tensor("yt", [128, PAIRS * R], f16, kind="ExternalOutput").ap()

    with tile.TileContext(nc) as tc, ExitStack() as ctx:
        const_pool = ctx.enter_context(tc.tile_pool(name="const", bufs=1))
        small_pool = ctx.enter_context(tc.tile_pool(name="small", bufs=1))
        psum_prep = ctx.enter_context(tc.tile_pool(name="pprep", bufs=2, space="PSUM"))
        psum_main = ctx.enter_context(tc.tile_pool(name="pmain", bufs=6, space="PSUM"))
        xpool = ctx.enter_context(tc.tile_pool(name="xin", bufs=PRE + 1))

        # ---- x load prefetch on the sync HWDGE ring (starts immediately)
        tiles = {}

        def load(k):
            x2 = xpool.tile([128, NB * R], f16)
            nc.sync.dma_start(x2[:], xt[:, k * NB * R : (k + 1) * NB * R])
            tiles[k] = x2

        for k in range(min(PRE, NT)):
            load(k)

        # ---- const loads on the scalar HWDGE ring (read/read mix with sync)
        ident_sb = const_pool.tile([128, 128], f32)
        nc.scalar.dma_start(ident_sb[:], ident[:])
        e1 = const_pool.tile([128, EMB], f32)
        nc.scalar.dma_start(e1[:], emb[0:128, :])
        e2 = const_pool.tile([80, EMB], f32, tag="e2")
        nc.scalar.dma_start(e2[:], emb[128:NODE_P, :])
        bias_sb = const_pool.tile([128, F], f32)
        nc.scalar.dma_start(bias_sb[:], bias_d[:])
        wsb = const_pool.tile([128, F * C], f16)
        nc.scalar.dma_start(wsb[:], w[:])

        # ---- stationary weights zero-fill (off-diagonal quadrants stay 0)
        Ew = const_pool.tile([128, 128 * PAIRS], f16)
        nc.gpsimd.memset(Ew[:], 0.0)

        # ---- E^T (unscaled) via PE transpose
        ET = small_pool.tile([128, NODE_P], f32)
        tp1 = psum_prep.tile([128, 128], f32, tag="prep")
        nc.tensor.transpose(tp1[:], e1[:], ident_sb[:])
        nc.vector.tensor_copy(ET[:, 0:128], tp1[:])
        tp2 = psum_prep.tile([128, 80], f32, tag="prep")
        nc.tensor.transpose(tp2[:], e2[:], ident_sb[0:80, 0:80])
        nc.vector.tensor_copy(ET[:, 128:NODE_P], tp2[:])

        # ---- sim = relu(E E^T) row-tile; d = 1 + softmax diag; E' = diag(d)E
        # diag(sim) needs no mask: relu(E E^T)[n,n] = ||E_n||^2 = rowsum(E^2)
        def diag_gate(rows, off, e_tile):
            simp_t = psum_prep.tile([128, NODE_P], f32, tag="prep")
            simp = simp_t[0:rows, :]
            nc.tensor.matmul(simp, ET[:, off : off + rows], ET[:])
            s_t = small_pool.tile([128, NODE_P], f32, tag=f"s{off}")
            s = s_t[0:rows, :]
            nc.vector.tensor_relu(s[:], simp[:])
            m_t = small_pool.tile([128, 1], f32, tag=f"m{off}")
            m = m_t[0:rows, :]
            nc.vector.tensor_reduce(m[:], s[:, 0:NODE], AX.X, ALU.max)
            negm_t = small_pool.tile([128, 1], f32, tag=f"negm{off}")
            negm = negm_t[0:rows, :]
            nc.vector.tensor_scalar_mul(negm[:], m[:], -1.0)
            ex_t = small_pool.tile([128, NODE], f32, tag=f"ex{off}")
            ex = ex_t[0:rows, :]
            nc.scalar.activation(ex[:], s[:, 0:NODE], AF.Exp, bias=negm[:])
            z_t = small_pool.tile([128, 1], f32, tag=f"z{off}")
            z = z_t[0:rows, :]
            nc.vector.tensor_reduce(z[:], ex[:], AX.X, ALU.add)
            sq_t = small_pool.tile([128, EMB], f32, tag=f"sq{off}")
            sq = sq_t[0:rows, :]
            nc.vector.tensor_mul(sq[:], e_tile[:], e_tile[:])
            dg_t = small_pool.tile([128, 1], f32, tag=f"dg{off}")
            dg = dg_t[0:rows, :]
            nc.vector.tensor_reduce(dg[:], sq[:], AX.X, ALU.add)
            ed_t = small_pool.tile([128, 1], f32, tag=f"ed{off}")
            ed = ed_t[0:rows, :]
            nc.scalar.activation(ed[:], dg[:], AF.Exp, bias=negm[:])
            rz_t = small_pool.tile([128, 1], f32, tag=f"rz{off}")
            rz = rz_t[0:rows, :]
            nc.vector.reciprocal(rz[:], z[:])
            d_t = small_pool.tile([128, 1], f32, tag=f"d{off}")
            d = d_t[0:rows, :]
            nc.vector.tensor_mul(d[:], ed[:], rz[:])
            nc.vector.tensor_scalar_add(d[:], d[:], 1.0)
            # E' = diag(d) @ E rows
            ep_t = small_pool.tile([128, EMB], f32, tag=f"ep{off}")
            ep = ep_t[0:rows, :]
            nc.vector.tensor_scalar_mul(ep[:], e_tile[:], d[:])
            return ep

        ep1 = diag_gate(128, 0, e1)
        ep2 = diag_gate(80, 128, e2)

        # ---- E'^T via PE transpose, cast to fp16 for the Ew matmuls
        EpT = small_pool.tile([128, NODE_P], f32)
        tq1 = psum_prep.tile([128, 128], f32, tag="prep")
        nc.tensor.transpose(tq1[:], ep1[:], ident_sb[:])
        nc.vector.tensor_copy(EpT[:, 0:128], tq1[:])
        tq2 = psum_prep.tile([128, 80], f32, tag="prep")
        nc.tensor.transpose(tq2[:], ep2[:], ident_sb[0:80, 0:80])
        nc.vector.tensor_copy(EpT[:, 128:NODE_P], tq2[:])
        EpT16 = small_pool.tile([128, NODE_P], f16)
        nc.vector.tensor_copy(EpT16[:], EpT[:])

        # ---- per-node bias, pair-stacked: ebT2[64*par + f, p] = eb[2p+par, f]
        ebT2 = const_pool.tile([128, PAIRS], f32)
        pe = psum_prep.tile([128, PAIRS], f32, tag="prep")
        nc.tensor.matmul(pe[0:64, :], bias_sb[:], ET[:, 0:NODE_P:2])
        nc.tensor.matmul(pe[64:128, :], bias_sb[:], ET[:, 1:NODE_P:2])
        nc.vector.tensor_copy(ebT2[:], pe[:])

        # ---- stationary weights, block-diagonal per pair (fp16):
        #   Ew[c,       p*128 + f]      = sum_d E'[2p,   d] W[d, c, f]
        #   Ew[64 + c,  p*128 + 64 + f] = sum_d E'[2p+1, d] W[d, c, f]
        # built 4 f-values per PSUM tile; copies land 4 strided f-columns at
        # once through the (p q b) view of Ew
        Ew3 = Ew[:].rearrange("p (q b) -> p q b", b=128)
        FC = 4
        for fc in range(F // FC):
            pf = psum_prep.tile([128, FC * PAIRS], f32, tag="prep")
            for ff in range(FC):
                f = fc * FC + ff
                wf = wsb[:, f * C : (f + 1) * C]  # W[:, :, f] (w is f-major)
                cols = slice(ff * PAIRS, (ff + 1) * PAIRS)
                nc.tensor.matmul(pf[0:64, cols], wf, EpT16[:, 0:NODE_P:2])
                nc.tensor.matmul(pf[64:128, cols], wf, EpT16[:, 1:NODE_P:2])
            src = pf[:].rearrange("p (b q) -> p q b", b=FC)
            f0 = fc * FC
            nc.vector.tensor_copy(Ew3[0:64, :, f0 : f0 + FC], src[0:64, :, :])
            nc.vector.tensor_copy(
                Ew3[64:128, :, 64 + f0 : 64 + f0 + FC], src[64:128, :, :]
            )

        # ---- main streaming loop: NT tiles of NB pairs
        # bias-add alternates Vector/Scalar (GpSimd has no PSUM port)
        def compute(k):
            x2 = tiles.pop(k)
            for j in range(NB):
                p = k * NB + j
                ew_p = Ew[:, p * 128 : (p + 1) * 128]
                for h in range(2):
                    ps = psum_main.tile([128, RH], f32)
                    cols = slice(j * R + h * RH, j * R + (h + 1) * RH)
                    nc.tensor.matmul(ps[:], ew_p, x2[:, cols])
                    if (j * 2 + h) % 2 == 0:
                        nc.vector.tensor_scalar_add(
                            x2[:, cols], ps[:], ebT2[:, p : p + 1]
                        )
                    else:
                        nc.scalar.activation(
                            x2[:, cols], ps[:], AF.Identity, bias=ebT2[:, p : p + 1]
                        )
            nc.sync.dma_start(yt[:, k * NB * R : (k + 1) * NB * R], x2[:])

        for k in range(NT):
            compute(k)
            if k + PRE < NT:
                load(k + PRE)

    nc.compile()
    return nc


def _get_nc():
    if "nc" not in _CACHE:
        _CACHE["nc"] = _build()
    return _CACHE["nc"]


def _host_prep(x, node_embedding, weights, bias):
    """Build per-core in_maps: fp16 pair-contiguous x, fp16 f-major W."""
    emb_p = np.zeros((NODE_P, EMB), np.float32)
    emb_p[:NODE] = node_embedding
    w2 = np.ascontiguousarray(
        weights.transpose(0, 2, 1).reshape(EMB, F * C), np.float16
    )
    bias_f = np.ascontiguousarray(bias, np.float32)
    ident_np = np.eye(128, dtype=np.float32)

    in_maps = []
    for i in range(N_CORES):
        xi = np.asarray(x[B_SH * i : B_SH * (i + 1)])  # [2, T, NODE, C]
        xp = np.zeros((B_SH, T, NODE_P, C), np.float16)
        xp[:, :, :NODE] = xi
        # xt[(par,c), p*R + (b,t)] = x[b, t, 2p+par, c]
        xt = (
            xp.reshape(B_SH, T, PAIRS, 2, C)
            .transpose(3, 4, 2, 0, 1)
            .reshape(128, PAIRS * R)
        )
        in_maps.append(
            {
                "xt": np.ascontiguousarray(xt),
                "emb": emb_p,
                "w": w2,
                "bias": bias_f,
                "ident": ident_np,
            }
        )
    return in_maps


def _host_post(results):
    out = np.empty((B, T, NODE, F), np.float32)
    for i in range(N_CORES):
        ytr = results[i]["yt"].reshape(2, F, PAIRS, B_SH, T)
        y_local = ytr.transpose(3, 4, 2, 0, 1).reshape(B_SH, T, NODE_P, F)
        out[B_SH * i : B_SH * (i + 1)] = y_local[:, :, :NODE, :].astype(np.float32)
    return out


def kernel(x, node_embedding, weights, bias):
    from concourse.bass_utils import run_bass_kernel_spmd

    nc = _get_nc()
    in_maps = _host_prep(x, node_embedding, weights, bias)
    res = run_bass_kernel_spmd(nc, in_maps, core_ids=list(range(N_CORES)))
    return _host_post(res.results)


# revision 6
# speedup vs baseline: 1.2378x; 1.2378x over previous
"""AdaptiveGraphConv Trainium2 kernel, data-parallel over batch on 8 NeuronCores.

Reference computation (per full input):
  sim  = relu(E @ E^T)                               [N, N]
  d[n] = 1 + softmax(sim, axis=1)[n, n]              (diag gate)
  Ew   = einsum('nd,dcf->ncf', E, W)                 per-node weights
  eb   = E @ bias                                    per-node bias [N, F]
  y[b,t,n,f] = (d[n] * x[b,t,n,:]) @ Ew[n] + eb[n]

This problem is memory-bound: the 15.6 GFLOP bulk is streaming x (31 MB/core
fp32) through per-node [64,64] matmuls. Strategy:

  - The tiny node-conditioned weight transform (d-gated Ew, eb: ~0.2 GFLOP
    total) is computed on the host in fp32 and shipped per-core as a
    block-diagonal fp16 stationary tensor `ew` (pairs of nodes -> [128,128]
    blocks) plus pair-stacked bias `ebt`. The device kernel is then a pure
    streaming pipeline with zero on-chip prep.
  - x/y stream through the chip in fp16 (tolerance 2e-2; fp16 error ~1e-3),
    halving HBM traffic. Host lays x out as [128, PAIRS*R]: partition
    q = (parity, c) of a node pair, column p*R + r, so an 8-pair tile is one
    9216B-contiguous descriptor per partition (small packets measurably cap
    SDMA engines well below line rate).
  - Device: ALL loads are issued up front on the sync HWDGE ring (13 tiles
    live in SBUF simultaneously), stores trail after each tile's compute, so
    ring traffic is reads-then-writes with a single turnaround and the DMA
    engines never idle. Per pair: 2 fp16 matmuls [128x128]@[128x288] into
    PSUM (8-bank rotation keeps the PE array continuously busy -> full
    2.4GHz p-state), then a PSUM->SBUF bias-add written in place over the x
    tile, alternating Vector/Scalar so neither engine bottlenecks.
  - host un-permutes y^T shards back to [B, T, N, F] and casts to fp32.
"""

import sys

sys.path.insert(0, "/opt/trn_rl_repo")

from contextlib import ExitStack

import numpy as np

N_CORES = 8
NODE = 207
NODE_P = 208  # padded to even node count
PAIRS = NODE_P // 2  # 104
EMB = 128
C = 64
F = 64
B = 16
T = 288
B_SH = B // N_CORES  # 2
R = B_SH * T  # 576 rows per core
RH = R // 2  # 288, matmul free-dim chunk
NB = 8  # pairs per DMA / compute tile
NT = PAIRS // NB  # 13 tiles
# ew arrives in chunks (multiples of NB pairs) interleaved with the first x
# loads so tile 0's compute isn't gated on the whole 3.4MB weight transfer
EW_CHUNKS = [16, 48, 40]

_CACHE = {}


def _build():
    import concourse.tile as tile
    from concourse import bacc, mybir

    f32 = mybir.dt.float32
    f16 = mybir.dt.float16
    AF = mybir.ActivationFunctionType

    nc = bacc.Bacc("TRN2", target_bir_lowering=False, debug=False, num_devices=N_CORES)
    xt = nc.dram_tensor("xt", [128, PAIRS * R], f16, kind="ExternalInput").ap()
    ew_d = nc.dram_tensor("ew", [128, 128 * PAIRS], f16, kind="ExternalInput").ap()
    ebt_d = nc.dram_tensor("ebt", [128, PAIRS], f32, kind="ExternalInput").ap()
    yt = nc.dram_tensor("yt", [128, PAIRS * R], f16, kind="ExternalOutput").ap()

    with tile.TileContext(nc) as tc, ExitStack() as ctx:
        const_pool = ctx.enter_context(tc.tile_pool(name="const", bufs=1))
        psum_main = ctx.enter_context(tc.tile_pool(name="pmain", bufs=8, space="PSUM"))
        xpool = ctx.enter_context(tc.tile_pool(name="xin", bufs=NT))

        ebt = const_pool.tile([128, PAIRS], f32)
        nc.sync.dma_start(ebt[:], ebt_d[:])
        Ew = const_pool.tile([128, 128 * PAIRS], f16)

        # all x loads up front, ew chunks woven between the first few so the
        # SDMA queue stays read-only until computes finish, then write-only
        tiles = []
        p0 = 0
        for k in range(NT):
            x2 = xpool.tile([128, NB * R], f16)
            nc.sync.dma_start(x2[:], xt[:, k * NB * R : (k + 1) * NB * R])
            tiles.append(x2)
            if k < len(EW_CHUNKS):
                cn = EW_CHUNKS[k]
                nc.sync.dma_start(
                    Ew[:, p0 * 128 : (p0 + cn) * 128],
                    ew_d[:, p0 * 128 : (p0 + cn) * 128],
                )
                p0 += cn

        # compute per tile; bias-add alternates Vector/Scalar; store trails
        for k in range(NT):
            x2 = tiles[k]
            for j in range(NB):
                p = k * NB + j
                ew_p = Ew[:, p * 128 : (p + 1) * 128]
                for h in range(2):
                    ps = psum_main.tile([128, RH], f32)
                    cols = slice(j * R + h * RH, j * R + (h + 1) * RH)
                    nc.tensor.matmul(ps[:], ew_p, x2[:, cols])
                    if (j * 2 + h) % 2 == 0:
                        nc.vector.tensor_scalar_add(
                            x2[:, cols], ps[:], ebt[:, p : p + 1]
                        )
                    else:
                        nc.scalar.activation(
                            x2[:, cols], ps[:], AF.Identity, bias=ebt[:, p : p + 1]
                        )
            nc.sync.dma_start(yt[:, k * NB * R : (k + 1) * NB * R], x2[:])

    nc.compile()
    return nc


def _get_nc():
    if "nc" not in _CACHE:
        _CACHE["nc"] = _build()
    return _CACHE["nc"]


def _host_prep(x, node_embedding, weights, bias):
    """Host side: node-conditioned weight transform (fp32, ~0.2 GFLOP) and
    per-core fp16 pair-contiguous x layout."""
    E = np.asarray(node_embedding, np.float32)  # [207, 128]
    W = np.asarray(weights, np.float32)  # [128, 64, 64]
    bias_f = np.asarray(bias, np.float32)  # [128, 64]

    # d[n] = 1 + softmax(relu(E E^T), axis=1)[n, n]
    sim = E @ E.T
    np.maximum(sim, 0.0, out=sim)
    m = sim.max(axis=1)
    ex = np.exp(sim - m[:, None])
    d = 1.0 + ex[np.arange(NODE), np.arange(NODE)] / ex.sum(axis=1)

    # per-node weights (d-gated) and bias
    EwN = (E @ W.reshape(EMB, C * F)).reshape(NODE, C, F) * d[:, None, None]
    ebN = E @ bias_f  # [207, 64]

    # pad to 208 nodes, pack pairs
    EwP = np.zeros((NODE_P, C, F), np.float32)
    EwP[:NODE] = EwN
    ebP = np.zeros((NODE_P, F), np.float32)
    ebP[:NODE] = ebN
    EwP = EwP.reshape(PAIRS, 2, C, F)
    ebP = ebP.reshape(PAIRS, 2, F)

    # block-diagonal stationary: ew[(par,c), p*128 + (par,f)] = EwP[p,par,c,f]
    ew_v = np.zeros((2, C, PAIRS, 2, F), np.float16)
    for par in range(2):
        ew_v[par, :, :, par, :] = EwP[:, par].transpose(1, 0, 2)
    ew = np.ascontiguousarray(ew_v.reshape(128, PAIRS * 128))
    # pair-stacked bias: ebt[par*64 + f, p] = ebP[p, par, f]
    ebt = np.ascontiguousarray(ebP.transpose(1, 2, 0).reshape(128, PAIRS))

    in_maps = []
    for i in range(N_CORES):
        xi = np.asarray(x[B_SH * i : B_SH * (i + 1)])  # [2, T, NODE, C]
        xp = np.zeros((B_SH, T, NODE_P, C), np.float16)
        xp[:, :, :NODE] = xi
        # xt[(par,c), p*R + (b,t)] = x[b, t, 2p+par, c]
        xt = (
            xp.reshape(B_SH, T, PAIRS, 2, C)
            .transpose(3, 4, 2, 0, 1)
            .reshape(128, PAIRS * R)
        )
        in_maps.append(
            {"xt": np.ascontiguousarray(xt), "ew": ew, "ebt": ebt}
        )
    return in_maps


def _host_post(results):
    out = np.empty((B, T, NODE, F), np.float32)
    for i in range(N_CORES):
        ytr = results[i]["yt"].reshape(2, F, PAIRS, B_SH, T)
        y_local = ytr.transpose(3, 4, 2, 0, 1).reshape(B_SH, T, NODE_P, F)
        out[B_SH * i : B_SH * (i + 1)] = y_local[:, :, :NODE, :].astype(np.float32)
    return out


def kernel(x, node_embedding, weights, bias):
    from concourse.bass_utils import run_bass_kernel_spmd

    nc = _get_nc()
    in_maps = _host_prep(x, node_embedding, weights, bias)
    res = run_bass_kernel_spmd(nc, in_maps, core_ids=list(range(N_CORES)))
    return _host_post(res.results)


# revision 9
# speedup vs baseline: 1.2956x; 1.0466x over previous
"""AdaptiveGraphConv Trainium2 kernel, data-parallel over batch on 8 NeuronCores.

Reference computation (per full input):
  sim  = relu(E @ E^T)                               [N, N]
  d[n] = 1 + softmax(sim, axis=1)[n, n]              (diag gate)
  Ew   = einsum('nd,dcf->ncf', E, W)                 per-node weights
  eb   = E @ bias                                    per-node bias [N, F]
  y[b,t,n,f] = (d[n] * x[b,t,n,:]) @ Ew[n] + eb[n]

This problem is memory-bound: the 15.6 GFLOP bulk is streaming x (31 MB/core
fp32) through per-node [64,64] matmuls. Strategy:

  - The tiny node-conditioned weight transform (d-gated Ew, eb: ~0.2 GFLOP
    total) is computed on the host in fp32 and shipped per-core as a
    block-diagonal fp16 stationary tensor `ew` (pairs of nodes -> [128,128]
    blocks) plus pair-stacked bias `ebt`. The device kernel is then a pure
    streaming pipeline with zero on-chip prep.
  - x/y stream through the chip in fp16 (tolerance 2e-2; fp16 error ~1e-3),
    halving HBM traffic. Host lays x out as [128, PAIRS*R]: partition
    q = (parity, c) of a node pair, column p*R + r, so an 8-pair tile is one
    9216B-contiguous descriptor per partition (small packets measurably cap
    SDMA engines well below line rate).
  - Device: ALL loads are issued up front on the sync HWDGE ring (13 tiles
    live in SBUF simultaneously), stores trail after each tile's compute, so
    ring traffic is reads-then-writes with a single turnaround and the DMA
    engines never idle. Per pair: 2 fp16 matmuls [128x128]@[128x288] into
    PSUM (8-bank rotation keeps the PE array continuously busy -> full
    2.4GHz p-state), then a PSUM->SBUF bias-add written in place over the x
    tile, alternating Vector/Scalar so neither engine bottlenecks.
  - host un-permutes y^T shards back to [B, T, N, F] and casts to fp32.
"""

import sys

sys.path.insert(0, "/opt/trn_rl_repo")

from contextlib import ExitStack

import numpy as np

N_CORES = 8
NODE = 207
NODE_P = 208  # padded to even node count
PAIRS = NODE_P // 2  # 104
EMB = 128
C = 64
F = 64
B = 16
T = 288
B_SH = B // N_CORES  # 2
R = B_SH * T  # 576 rows per core
RH = R // 2  # 288, matmul free-dim chunk
NB = 8  # pairs per DMA / compute tile
NT = PAIRS // NB  # 13 tiles
# ew arrives in chunks (multiples of NB pairs) interleaved with the first x
# loads so tile 0's compute isn't gated on the whole 3.4MB weight transfer
EW_CHUNKS = [16, 48, 40]

_CACHE = {}


def _build():
    import concourse.tile as tile
    from concourse import bacc, mybir

    f32 = mybir.dt.float32
    f16 = mybir.dt.float16
    AF = mybir.ActivationFunctionType

    nc = bacc.Bacc("TRN2", target_bir_lowering=False, debug=False, num_devices=N_CORES)
    xt = nc.dram_tensor("xt", [128, PAIRS * R], f16, kind="ExternalInput").ap()
    ewc_d = nc.dram_tensor("ewc", [128, F * PAIRS], f16, kind="ExternalInput").ap()
    ebt_d = nc.dram_tensor("ebt", [128, PAIRS], f32, kind="ExternalInput").ap()
    yt = nc.dram_tensor("yt", [128, PAIRS * R], f16, kind="ExternalOutput").ap()

    with tile.TileContext(nc) as tc, ExitStack() as ctx:
        const_pool = ctx.enter_context(tc.tile_pool(name="const", bufs=1))
        psum_main = ctx.enter_context(tc.tile_pool(name="pmain", bufs=8, space="PSUM"))
        xpool = ctx.enter_context(tc.tile_pool(name="xin", bufs=NT))

        ebt = const_pool.tile([128, PAIRS], f32)
        Ew = const_pool.tile([128, 128 * PAIRS], f16)
        ewc = const_pool.tile([128, F * PAIRS], f16)
        Ew3 = Ew[:].rearrange("q (p g) -> q p g", g=128)
        ewc3 = ewc[:].rearrange("q (p g) -> q p g", g=F)

        # all x loads up front; the compact weights (half the bytes of the
        # block-diagonal form) are woven between the first few loads and
        # expanded on-chip by the otherwise-idle Vector/Scalar/GpSimd engines
        tiles = []
        p0 = 0
        for k in range(NT):
            x2 = xpool.tile([128, NB * R], f16)
            nc.sync.dma_start(x2[:], xt[:, k * NB * R : (k + 1) * NB * R])
            tiles.append(x2)
            if k == 0:
                nc.sync.dma_start(ebt[:], ebt_d[:])
            if k < len(EW_CHUNKS):
                cn = EW_CHUNKS[k]
                nc.sync.dma_start(
                    ewc[:, p0 * F : (p0 + cn) * F],
                    ewc_d[:, p0 * F : (p0 + cn) * F],
                )
                # expand to block-diagonal: data quadrants + zero quadrants
                pp = slice(p0, p0 + cn)
                nc.vector.tensor_copy(Ew3[0:64, pp, 0:64], ewc3[0:64, pp, :])
                nc.vector.memset(Ew3[0:64, pp, 64:128], 0.0)
                nc.scalar.activation(
                    Ew3[64:128, pp, 64:128], ewc3[64:128, pp, :], AF.Identity
                )
                nc.gpsimd.memset(Ew3[64:128, pp, 0:64], 0.0)
                p0 += cn

        # compute per tile; bias-add alternates Vector/Scalar; store trails
        for k in range(NT):
            x2 = tiles[k]
            for j in range(NB):
                p = k * NB + j
                ew_p = Ew[:, p * 128 : (p + 1) * 128]
                for h in range(2):
                    ps = psum_main.tile([128, RH], f32)
                    cols = slice(j * R + h * RH, j * R + (h + 1) * RH)
                    nc.tensor.matmul(ps[:], ew_p, x2[:, cols])
                    if (j * 2 + h) % 2 == 0:
                        nc.vector.tensor_scalar_add(
                            x2[:, cols], ps[:], ebt[:, p : p + 1]
                        )
                    else:
                        nc.scalar.activation(
                            x2[:, cols], ps[:], AF.Identity, bias=ebt[:, p : p + 1]
                        )
            nc.sync.dma_start(yt[:, k * NB * R : (k + 1) * NB * R], x2[:])

    nc.compile()
    return nc


def _get_nc():
    if "nc" not in _CACHE:
        _CACHE["nc"] = _build()
    return _CACHE["nc"]


def _host_prep(x, node_embedding, weights, bias):
    """Host side: node-conditioned weight transform (fp32, ~0.2 GFLOP) and
    per-core fp16 pair-contiguous x layout."""
    E = np.asarray(node_embedding, np.float32)  # [207, 128]
    W = np.asarray(weights, np.float32)  # [128, 64, 64]
    bias_f = np.asarray(bias, np.float32)  # [128, 64]

    # d[n] = 1 + softmax(relu(E E^T), axis=1)[n, n]
    sim = E @ E.T
    np.maximum(sim, 0.0, out=sim)
    m = sim.max(axis=1)
    ex = np.exp(sim - m[:, None])
    d = 1.0 + ex[np.arange(NODE), np.arange(NODE)] / ex.sum(axis=1)

    # per-node weights (d-gated) and bias
    EwN = (E @ W.reshape(EMB, C * F)).reshape(NODE, C, F) * d[:, None, None]
    ebN = E @ bias_f  # [207, 64]

    # pad to 208 nodes, pack pairs
    EwP = np.zeros((NODE_P, C, F), np.float32)
    EwP[:NODE] = EwN
    ebP = np.zeros((NODE_P, F), np.float32)
    ebP[:NODE] = ebN
    EwP = EwP.reshape(PAIRS, 2, C, F)
    ebP = ebP.reshape(PAIRS, 2, F)

    # compact stationary: ewc[(par,c), p*64 + f] = EwP[p, par, c, f]
    # (device expands to the block-diagonal [128,128]-per-pair form)
    ewc = np.ascontiguousarray(
        EwP.transpose(1, 2, 0, 3).astype(np.float16).reshape(128, PAIRS * F)
    )
    # pair-stacked bias: ebt[par*64 + f, p] = ebP[p, par, f]
    ebt = np.ascontiguousarray(ebP.transpose(1, 2, 0).reshape(128, PAIRS))

    in_maps = []
    for i in range(N_CORES):
        xi = np.asarray(x[B_SH * i : B_SH * (i + 1)])  # [2, T, NODE, C]
        xp = np.zeros((B_SH, T, NODE_P, C), np.float16)
        xp[:, :, :NODE] = xi
        # xt[(par,c), p*R + (b,t)] = x[b, t, 2p+par, c]
        xt = (
            xp.reshape(B_SH, T, PAIRS, 2, C)
            .transpose(3, 4, 2, 0, 1)
            .reshape(128, PAIRS * R)
        )
        in_maps.append(
            {"xt": np.ascontiguousarray(xt), "ewc": ewc, "ebt": ebt}
        )
    return in_maps


def _host_post(results):
    out = np.empty((B, T, NODE, F), np.float32)
    for i in range(N_CORES):
        ytr = results[i]["yt"].reshape(2, F, PAIRS, B_SH, T)
        y_local = ytr.transpose(3, 4, 2, 0, 1).reshape(B_SH, T, NODE_P, F)
        out[B_SH * i : B_SH * (i + 1)] = y_local[:, :, :NODE, :].astype(np.float32)
    return out


def kernel(x, node_embedding, weights, bias):
    from concourse.bass_utils import run_bass_kernel_spmd

    nc = _get_nc()
    in_maps = _host_prep(x, node_embedding, weights, bias)
    res = run_bass_kernel_spmd(nc, in_maps, core_ids=list(range(N_CORES)))
    return _host_post(res.results)


# revision 10
# speedup vs baseline: 1.3044x; 1.0068x over previous
"""AdaptiveGraphConv Trainium2 kernel, data-parallel over batch on 8 NeuronCores.

Reference computation (per full input):
  sim  = relu(E @ E^T)                               [N, N]
  d[n] = 1 + softmax(sim, axis=1)[n, n]              (diag gate)
  Ew   = einsum('nd,dcf->ncf', E, W)                 per-node weights
  eb   = E @ bias                                    per-node bias [N, F]
  y[b,t,n,f] = (d[n] * x[b,t,n,:]) @ Ew[n] + eb[n]

This problem is memory-bound: the 15.6 GFLOP bulk is streaming x (31 MB/core
fp32) through per-node [64,64] matmuls. Strategy:

  - The tiny node-conditioned weight transform (d-gated Ew, eb: ~0.2 GFLOP
    total) is computed on the host in fp32 and shipped per-core as a
    block-diagonal fp16 stationary tensor `ew` (pairs of nodes -> [128,128]
    blocks) plus pair-stacked bias `ebt`. The device kernel is then a pure
    streaming pipeline with zero on-chip prep.
  - x/y stream through the chip in fp16 (tolerance 2e-2; fp16 error ~1e-3),
    halving HBM traffic. Host lays x out as [128, PAIRS*R]: partition
    q = (parity, c) of a node pair, column p*R + r, so an 8-pair tile is one
    9216B-contiguous descriptor per partition (small packets measurably cap
    SDMA engines well below line rate).
  - Device: ALL loads are issued up front on the sync HWDGE ring (13 tiles
    live in SBUF simultaneously), stores trail after each tile's compute, so
    ring traffic is reads-then-writes with a single turnaround and the DMA
    engines never idle. Per pair: 2 fp16 matmuls [128x128]@[128x288] into
    PSUM (8-bank rotation keeps the PE array continuously busy -> full
    2.4GHz p-state), then a PSUM->SBUF bias-add written in place over the x
    tile, alternating Vector/Scalar so neither engine bottlenecks.
  - host un-permutes y^T shards back to [B, T, N, F] and casts to fp32.
"""

import sys

sys.path.insert(0, "/opt/trn_rl_repo")

from contextlib import ExitStack

import numpy as np

N_CORES = 8
NODE = 207
NODE_P = 208  # padded to even node count
PAIRS = NODE_P // 2  # 104
EMB = 128
C = 64
F = 64
B = 16
T = 288
B_SH = B // N_CORES  # 2
R = B_SH * T  # 576 rows per core
RH = R // 2  # 288, matmul free-dim chunk
NB = 13  # pairs per DMA / compute tile
NT = PAIRS // NB  # 8 tiles
# ew arrives in chunks (multiples of NB pairs) interleaved with the first x
# loads so tile 0's compute isn't gated on the whole weight transfer
EW_CHUNKS = [26, 39, 39]

_CACHE = {}


def _build():
    import concourse.tile as tile
    from concourse import bacc, mybir

    f32 = mybir.dt.float32
    f16 = mybir.dt.float16
    AF = mybir.ActivationFunctionType

    nc = bacc.Bacc("TRN2", target_bir_lowering=False, debug=False, num_devices=N_CORES)
    xt = nc.dram_tensor("xt", [128, PAIRS * R], f16, kind="ExternalInput").ap()
    ewc_d = nc.dram_tensor("ewc", [128, F * PAIRS], f16, kind="ExternalInput").ap()
    ebt_d = nc.dram_tensor("ebt", [128, PAIRS], f32, kind="ExternalInput").ap()
    yt = nc.dram_tensor("yt", [128, PAIRS * R], f16, kind="ExternalOutput").ap()

    with tile.TileContext(nc) as tc, ExitStack() as ctx:
        const_pool = ctx.enter_context(tc.tile_pool(name="const", bufs=1))
        psum_main = ctx.enter_context(tc.tile_pool(name="pmain", bufs=8, space="PSUM"))
        xpool = ctx.enter_context(tc.tile_pool(name="xin", bufs=NT))

        ebt = const_pool.tile([128, PAIRS], f32)
        Ew = const_pool.tile([128, 128 * PAIRS], f16)
        ewc = const_pool.tile([128, F * PAIRS], f16)
        Ew3 = Ew[:].rearrange("q (p g) -> q p g", g=128)
        ewc3 = ewc[:].rearrange("q (p g) -> q p g", g=F)

        # all x loads up front; the compact weights (half the bytes of the
        # block-diagonal form) are woven between the first few loads and
        # expanded on-chip by the otherwise-idle Vector/Scalar/GpSimd engines
        tiles = []
        p0 = 0
        for k in range(NT):
            x2 = xpool.tile([128, NB * R], f16)
            nc.sync.dma_start(x2[:], xt[:, k * NB * R : (k + 1) * NB * R])
            tiles.append(x2)
            if k == 0:
                nc.sync.dma_start(ebt[:], ebt_d[:])
            if k < len(EW_CHUNKS):
                cn = EW_CHUNKS[k]
                nc.sync.dma_start(
                    ewc[:, p0 * F : (p0 + cn) * F],
                    ewc_d[:, p0 * F : (p0 + cn) * F],
                )
                # expand to block-diagonal: data quadrants + zero quadrants
                pp = slice(p0, p0 + cn)
                nc.vector.tensor_copy(Ew3[0:64, pp, 0:64], ewc3[0:64, pp, :])
                nc.vector.memset(Ew3[0:64, pp, 64:128], 0.0)
                nc.scalar.activation(
                    Ew3[64:128, pp, 64:128], ewc3[64:128, pp, :], AF.Identity
                )
                nc.gpsimd.memset(Ew3[64:128, pp, 0:64], 0.0)
                p0 += cn

        # compute per tile; bias-add alternates Vector/Scalar; store trails
        for k in range(NT):
            x2 = tiles[k]
            for j in range(NB):
                p = k * NB + j
                ew_p = Ew[:, p * 128 : (p + 1) * 128]
                for h in range(2):
                    ps = psum_main.tile([128, RH], f32)
                    cols = slice(j * R + h * RH, j * R + (h + 1) * RH)
                    nc.tensor.matmul(ps[:], ew_p, x2[:, cols])
                    if (j * 2 + h) % 2 == 0:
                        nc.vector.tensor_scalar_add(
                            x2[:, cols], ps[:], ebt[:, p : p + 1]
                        )
                    else:
                        nc.scalar.activation(
                            x2[:, cols], ps[:], AF.Identity, bias=ebt[:, p : p + 1]
                        )
            nc.sync.dma_start(yt[:, k * NB * R : (k + 1) * NB * R], x2[:])

    nc.compile()
    return nc


def _get_nc():
    if "nc" not in _CACHE:
        _CACHE["nc"] = _build()
    return _CACHE["nc"]


def _host_prep(x, node_embedding, weights, bias):
    """Host side: node-conditioned weight transform (fp32, ~0.2 GFLOP) and
    per-core fp16 pair-contiguous x layout."""
    E = np.asarray(node_embedding, np.float32)  # [207, 128]
    W = np.asarray(weights, np.float32)  # [128, 64, 64]
    bias_f = np.asarray(bias, np.float32)  # [128, 64]

    # d[n] = 1 + softmax(relu(E E^T), axis=1)[n, n]
    sim = E @ E.T
    np.maximum(sim, 0.0, out=sim)
    m = sim.max(axis=1)
    ex = np.exp(sim - m[:, None])
    d = 1.0 + ex[np.arange(NODE), np.arange(NODE)] / ex.sum(axis=1)

    # per-node weights (d-gated) and bias
    EwN = (E @ W.reshape(EMB, C * F)).reshape(NODE, C, F) * d[:, None, None]
    ebN = E @ bias_f  # [207, 64]

    # pad to 208 nodes, pack pairs
    EwP = np.zeros((NODE_P, C, F), np.float32)
    EwP[:NODE] = EwN
    ebP = np.zeros((NODE_P, F), np.float32)
    ebP[:NODE] = ebN
    EwP = EwP.reshape(PAIRS, 2, C, F)
    ebP = ebP.reshape(PAIRS, 2, F)

    # compact stationary: ewc[(par,c), p*64 + f] = EwP[p, par, c, f]
    # (device expands to the block-diagonal [128,128]-per-pair form)
    ewc = np.ascontiguousarray(
        EwP.transpose(1, 2, 0, 3).astype(np.float16).reshape(128, PAIRS * F)
    )
    # pair-stacked bias: ebt[par*64 + f, p] = ebP[p, par, f]
    ebt = np.ascontiguousarray(ebP.transpose(1, 2, 0).reshape(128, PAIRS))

    in_maps = []
    for i in range(N_CORES):
        xi = np.asarray(x[B_SH * i : B_SH * (i + 1)])  # [2, T, NODE, C]
        xp = np.zeros((B_SH, T, NODE_P, C), np.float16)
        xp[:, :, :NODE] = xi
        # xt[(par,c), p*R + (b,t)] = x[b, t, 2p+par, c]
        xt = (
            xp.reshape(B_SH, T, PAIRS, 2, C)
            .transpose(3, 4, 2, 0, 1)
            .reshape(128, PAIRS * R)
        )
        in_maps.append(
            {"xt": np.ascontiguousarray(xt), "ewc": ewc, "ebt": ebt}
        )
    return in_maps


def _host_post(results):
    out = np.empty((B, T, NODE, F), np.float32)
    for i in range(N_CORES):
        ytr = results[i]["yt"].reshape(2, F, PAIRS, B_SH, T)
        y_local = ytr.transpose(3, 4, 2, 0, 1).reshape(B_SH, T, NODE_P, F)
        out[B_SH * i : B_SH * (i + 1)] = y_local[:, :, :NODE, :].astype(np.float32)
    return out


def kernel(x, node_embedding, weights, bias):
    from concourse.bass_utils import run_bass_kernel_spmd

    nc = _get_nc()
    in_maps = _host_prep(x, node_embedding, weights, bias)
    res = run_bass_kernel_spmd(nc, in_maps, core_ids=list(range(N_CORES)))
    return _host_post(res.results)
